# revision 14
# baseline (speedup 1.0000x reference)
"""Trainium2 Bass kernel for cross-attention (cosine-normalized, 8 heads).

Reference computation (full inputs x,y [1,4096,64]):
  q = x@Wq+bq ; k,v = split(y@Wkv+bkv) ; per head (8 heads, dim 8):
  attn = softmax(l2norm(q) @ l2norm(k)^T) ; out = attn@v
  result = concat_heads(out) @ We + be

Sharding: one head per NeuronCore (8 heads / 8 cores), SPMD program with
per-core weight slices. Each core returns resT_h = (out_h @ We_h + be/8)^T
as [64, 4096]; the host sums over cores and transposes.

Device algorithm per core (head h):
  - host passes xTe=[x^T; 1] [65,4096] (ones row folds biases into matmuls),
    yTe likewise, weight slices with bias rows, and a selector constant.
  - qT [8,4096] = Wqe^T @ xTe ; kT likewise (PE, K=65, fp32r).
  - norms in free layout: squares on DVE; selector matmuls pack per-block
    column sums into one [8,512] psum; invsqrt = newton(recip(sqrt)));
    flatten via DMA, replicate via gpsimd partition_broadcast; qT/kT
    normalized into fp32r tiles.
  - v chunks [128,8] = yTe_chunk^T @ Wve stored bf16 with ones column
    (vext [128, 9*32]); the ones column produces the softmax denominator.
  - main loop (8 q-blocks x 16 chunk-groups):
      scores^T [128,1024] = kTn_chunks^T qTn_block (fp32r, two matmuls)
      expS bf16 = Exp(scores) on ScalarE (cosine scores in [-1,1]: no
        max-subtraction needed)
      po [9,512] += vext_chunk^T @ expS  (rows 0-7 numerator, row 8 denom)
  - tail: invden = recip_accurate(den); oTe (incl. denom row) scaled by
    broadcast invden -> row 8 becomes exactly 1 = bias row for the final
    fp32 projection resT = WeBe^T @ oTe; DMA out.
"""

import sys

import numpy as np

for _p in ("/opt/trn_rl_repo",):
    if _p not in sys.path:
        sys.path.insert(0, _p)

from contextlib import ExitStack

import concourse.bass as bass
import concourse.tile as tile
from concourse import bacc, mybir
from concourse.bass import ts
from concourse.bass_utils import run_bass_kernel_spmd

F32 = mybir.dt.float32
F32R = mybir.dt.float32r
BF16 = mybir.dt.bfloat16

HW = 4096          # sequence length
C = 64             # model dim
H = 8              # heads
D = 8              # head dim
CE = C + 1         # +ones row for bias folding
QB = 512           # q block
NQB = HW // QB     # 8
KC = 128           # k chunk
NKC = HW // KC     # 32
GRP = 2            # k-chunks per exp/ACT group
NG = NKC // GRP    # 16
VW = D + 1         # v + ones column

REPL = "dma"        # inv replication: gpsimd partition_broadcast vs row DMAs

_BUILT = None
TRACE = False
LAST_RESULTS = None


def _body(ctx, tc, dram):
    nc = tc.nc
    xTe_d, yTe_d, wqe_d, wke_d, wve_d, webe_d, sel_d, out_d = dram

    if REPL == "pb":
        from concourse import library_config
        nc.gpsimd.load_library(library_config.attn)

    const = ctx.enter_context(tc.tile_pool(name="const", bufs=1))
    expp = ctx.enter_context(tc.tile_pool(name="exps", bufs=3))
    ps_s = ctx.enter_context(tc.tile_pool(name="ps_s", bufs=3, space="PSUM"))
    ps_o = ctx.enter_context(tc.tile_pool(name="ps_o", bufs=2, space="PSUM"))

    # ---------------- loads ----------------
    xTe = const.tile([CE, HW], F32R)
    nc.sync.dma_start(xTe[:], xTe_d)
    yTe = const.tile([CE, HW], F32R)
    nc.sync.dma_start(yTe[:], yTe_d)
    wqe = const.tile([CE, D], F32R)
    nc.sync.dma_start(wqe[:], wqe_d)
    wke = const.tile([CE, D], F32R)
    nc.sync.dma_start(wke[:], wke_d)
    wve = const.tile([CE, D], F32R)
    nc.sync.dma_start(wve[:], wve_d)
    webe = const.tile([VW, C], F32)
    nc.sync.dma_start(webe[:], webe_d)
    sel = const.tile([D, D * NQB], F32R)
    nc.sync.dma_start(sel[:], sel_d)

    # ---------------- q/k projections (transposed layout) ----------------
    qT = const.tile([D, HW], F32)
    kT = const.tile([D, HW], F32)
    for dst, w, src in ((qT, wqe, xTe), (kT, wke, yTe)):
        for j in range(NQB):
            ps = ps_s.tile([D, QB], F32, tag="pss")
            nc.tensor.matmul(ps[:], w[:], src[:, ts(j, QB)], start=True, stop=True)
            nc.vector.tensor_copy(dst[:, ts(j, QB)], ps[:])

    # ---------------- v prep (row layout, bf16, ones col) ----------------
    vext = const.tile([KC, VW * NKC], BF16)
    nc.vector.memset(vext[:], 1.0)
    for c in range(NKC):
        ps = ps_s.tile([KC, D], F32, tag="pss")
        nc.tensor.matmul(ps[:], yTe[:, ts(c, KC)], wve[:], start=True, stop=True)
        nc.vector.tensor_copy(vext[:, c * VW:c * VW + D], ps[:])

    # ---------------- inverse norms (free layout, partition-packed) -------
    # qTn doubles as the squares scratch before it holds normalized q.
    qTn = const.tile([D, HW], F32R)
    kTn = const.tile([D, HW], F32R)
    ssq_q = const.tile([NQB, QB], F32)
    ssq_k = const.tile([NQB, QB], F32)
    for src, ssq in ((qT, ssq_q), (kT, ssq_k)):
        nc.vector.tensor_mul(qTn[:], src[:], src[:])
        ps = ps_s.tile([NQB, QB], F32, tag="pss")
        for j in range(NQB):
            nc.tensor.matmul(ps[:], sel[:, ts(j, D)], qTn[:, ts(j, QB)],
                             start=(j == 0), stop=(j == NQB - 1))
        nc.vector.tensor_copy(ssq[:], ps[:])

    # invsqrt = newton(recip_accurate(sqrt(ssq)))
    inv_q = const.tile([NQB, QB], F32)
    inv_k = const.tile([NQB, QB], F32)
    scr = const.tile([NQB, QB], F32)
    scr2 = const.tile([NQB, QB], F32)
    for ssq, inv in ((ssq_q, inv_q), (ssq_k, inv_k)):
        nc.scalar.sqrt(scr[:], ssq[:])
        nc.vector.reciprocal_approx_accurate(inv[:], scr[:], scr2[:])
        nc.vector.tensor_mul(scr[:], inv[:], inv[:])
        nc.vector.tensor_mul(scr[:], scr[:], ssq[:])
        nc.vector.tensor_scalar(scr[:], scr[:], -0.5, 1.5,
                                mybir.AluOpType.mult, mybir.AluOpType.add)
        nc.vector.tensor_mul(inv[:], inv[:], scr[:])

    # replicate inv norms to D partitions; normalize into fp32r tiles
    rep = const.tile([D, HW], F32)
    invf = const.tile([1, HW], F32)
    for inv, src, dst in ((inv_q, qT, qTn), (inv_k, kT, kTn)):
        nc.sync.dma_start(invf[:], inv[:])  # [NQB, QB] -> [1, HW]
        if REPL == "pb":
            nc.gpsimd.partition_broadcast(rep[:], invf[:], channels=D)
        else:
            for p in range(D):
                nc.sync.dma_start(rep[p:p + 1, :], invf[:])
        nc.vector.tensor_mul(dst[:], src[:], rep[:])

    # ---------------- main attention loop ----------------
    # oTe rows 0-7: unnormalized numerator; row 8: softmax denominator
    # (after scaling by the replicated reciprocal, row 8 becomes den/den = 1,
    # which is exactly the ones-row the output projection needs for be/8).
    oTe = const.tile([VW, HW], F32)

    for j in range(NQB):
        po = ps_o.tile([VW, QB], F32, tag="pso")
        for g in range(NG):
            ps = ps_s.tile([KC, GRP * QB], F32, tag="pss")
            for u in range(GRP):
                c = g * GRP + u
                nc.tensor.matmul(ps[:, ts(u, QB)], kTn[:, ts(c, KC)],
                                 qTn[:, ts(j, QB)], start=True, stop=True)
            es = expp.tile([KC, GRP * QB], BF16, tag="es")
            nc.scalar.activation(es[:], ps[:], mybir.ActivationFunctionType.Exp)
            for u in range(GRP):
                c = g * GRP + u
                nc.tensor.matmul(po[:], vext[:, c * VW:(c + 1) * VW],
                                 es[:, ts(u, QB)],
                                 start=(g == 0 and u == 0),
                                 stop=(g == NG - 1 and u == GRP - 1))
        nc.vector.tensor_copy(oTe[:, ts(j, QB)], po[:])

    # ---------------- normalize + output projection ----------------
    # repack denominator row [1, HW] -> [NQB, QB] via DMA (partition crossing)
    den8 = const.tile([NQB, QB], F32)
    nc.sync.dma_start(den8[:], oTe[D:D + 1, :])
    invd = const.tile([NQB, QB], F32)
    scr3 = const.tile([NQB, QB], F32)
    nc.vector.reciprocal_approx_accurate(invd[:], den8[:], scr3[:])
    nc.sync.dma_start(invf[:], invd[:])
    repd = const.tile([VW, HW], F32)
    if REPL == "pb":
        nc.gpsimd.partition_broadcast(repd[:], invf[:], channels=VW)
    else:
        for p in range(VW):
            nc.sync.dma_start(repd[p:p + 1, :], invf[:])
    nc.vector.tensor_mul(oTe[:], oTe[:], repd[:])

    resT = const.tile([C, HW], F32)
    for j in range(NQB):
        ps = ps_s.tile([C, QB], F32, tag="pss")
        nc.tensor.matmul(ps[:], webe[:], oTe[:, ts(j, QB)], start=True, stop=True)
        nc.vector.tensor_copy(resT[:, ts(j, QB)], ps[:])
    nc.sync.dma_start(out_d, resT[:])


def _build():
    global _BUILT
    if _BUILT is not None:
        return _BUILT
    nc = bacc.Bacc("TRN2", target_bir_lowering=False, debug=False, num_devices=H)
    xTe_d = nc.dram_tensor("xTe", [CE, HW], F32R, kind="ExternalInput").ap()
    yTe_d = nc.dram_tensor("yTe", [CE, HW], F32R, kind="ExternalInput").ap()
    wqe_d = nc.dram_tensor("wqe", [CE, D], F32R, kind="ExternalInput").ap()
    wke_d = nc.dram_tensor("wke", [CE, D], F32R, kind="ExternalInput").ap()
    wve_d = nc.dram_tensor("wve", [CE, D], F32R, kind="ExternalInput").ap()
    webe_d = nc.dram_tensor("webe", [VW, C], F32, kind="ExternalInput").ap()
    sel_d = nc.dram_tensor("sel", [D, D * NQB], F32R, kind="ExternalInput").ap()
    out_d = nc.dram_tensor("resT", [C, HW], F32, kind="ExternalOutput").ap()
    with tile.TileContext(nc) as tc, ExitStack() as ctx:
        _body(ctx, tc, (xTe_d, yTe_d, wqe_d, wke_d, wve_d, webe_d, sel_d,
                        out_d[:]))
    nc.compile()
    _BUILT = nc
    return nc


def make_in_maps(x, y, Wq, bq, Wkv, bkv, We, be):
    x, y, Wq, bq, Wkv, bkv, We, be = (
        np.asarray(a, np.float32) for a in (x, y, Wq, bq, Wkv, bkv, We, be))
    ones = np.ones((1, HW), np.float32)
    xTe = np.ascontiguousarray(np.vstack([x[0].T, ones]))
    yTe = np.ascontiguousarray(np.vstack([y[0].T, ones]))
    sel = np.zeros((D, D * NQB), np.float32)
    for j in range(NQB):
        sel[:, D * j + j] = 1.0
    in_maps = []
    for h in range(H):
        sl = slice(h * D, (h + 1) * D)
        slv = slice(C + h * D, C + (h + 1) * D)
        in_maps.append({
            "xTe": xTe,
            "yTe": yTe,
            "wqe": np.ascontiguousarray(np.vstack([Wq[:, sl], bq[None, sl]])),
            "wke": np.ascontiguousarray(np.vstack([Wkv[:, sl], bkv[None, sl]])),
            "wve": np.ascontiguousarray(np.vstack([Wkv[:, slv], bkv[None, slv]])),
            "webe": np.ascontiguousarray(np.vstack([We[sl, :], be[None, :] / H])),
            "sel": sel,
        })
    return in_maps


def kernel(x, y, Wq, bq, Wkv, bkv, We, be):
    global LAST_RESULTS
    nc = _build()
    in_maps = make_in_maps(x, y, Wq, bq, Wkv, bkv, We, be)
    res = run_bass_kernel_spmd(nc, in_maps, core_ids=list(range(H)), trace=TRACE)
    LAST_RESULTS = res
    acc = np.zeros((C, HW), np.float64)
    for r in res.results:
        acc += r["resT"]
    return np.ascontiguousarray(acc.T[None]).astype(np.float32)


# revision 15
# speedup vs baseline: 1.0647x; 1.0647x over previous
"""Trainium2 Bass kernel for cross-attention (cosine-normalized, 8 heads).

Reference computation (full inputs x,y [1,4096,64]):
  q = x@Wq+bq ; k,v = split(y@Wkv+bkv) ; per head (8 heads, dim 8):
  attn = softmax(l2norm(q) @ l2norm(k)^T) ; out = attn@v
  result = concat_heads(out) @ We + be

Sharding: one head per NeuronCore (8 heads / 8 cores), SPMD program with
per-core weight slices. Each core returns resT_h = (out_h @ We_h + be/8)^T
as [64, 4096]; the host sums over cores and transposes.

Device algorithm per core (head h):
  - host passes xTe=[x^T; 1] [65,4096] (ones row folds biases into matmuls),
    yTe likewise, weight slices with bias rows, and a selector constant.
  - qT [8,4096] = Wqe^T @ xTe ; kT likewise (PE, K=65, fp32r).
  - norms in free layout: squares on DVE; selector matmuls pack per-block
    column sums into one [8,512] psum; invsqrt = newton(recip(sqrt)));
    flatten via DMA, replicate via gpsimd partition_broadcast; qT/kT
    normalized into fp32r tiles.
  - v chunks [128,8] = yTe_chunk^T @ Wve stored bf16 with ones column
    (vext [128, 9*32]); the ones column produces the softmax denominator.
  - main loop (8 q-blocks x 16 chunk-groups):
      scores^T [128,1024] = kTn_chunks^T qTn_block (fp32r, two matmuls)
      expS bf16 = Exp(scores) on ScalarE (cosine scores in [-1,1]: no
        max-subtraction needed)
      po [9,512] += vext_chunk^T @ expS  (rows 0-7 numerator, row 8 denom)
  - tail: invden = recip_accurate(den); oTe (incl. denom row) scaled by
    broadcast invden -> row 8 becomes exactly 1 = bias row for the final
    fp32 projection resT = WeBe^T @ oTe; DMA out.
"""

import sys

import numpy as np

for _p in ("/opt/trn_rl_repo",):
    if _p not in sys.path:
        sys.path.insert(0, _p)

from contextlib import ExitStack

import concourse.bass as bass
import concourse.tile as tile
from concourse import bacc, mybir
from concourse.bass import ts
from concourse.bass_utils import run_bass_kernel_spmd

F32 = mybir.dt.float32
F32R = mybir.dt.float32r
BF16 = mybir.dt.bfloat16

HW = 4096          # sequence length
C = 64             # model dim
H = 8              # heads
D = 8              # head dim
CE = C + 1         # +ones row for bias folding
QB = 512           # q block
NQB = HW // QB     # 8
KC = 128           # k chunk
NKC = HW // KC     # 32
GRP = 2            # k-chunks per exp/ACT group
NG = NKC // GRP    # 16
VW = D + 1         # v + ones column

REPL = "dma"        # inv replication: gpsimd partition_broadcast vs row DMAs

_BUILT = None
TRACE = False
LAST_RESULTS = None


def _body(ctx, tc, dram):
    nc = tc.nc
    xTe_d, yTe_d, wqe_d, wke_d, wve_d, webe_d, sel_d, out_d = dram

    if REPL == "pb":
        from concourse import library_config
        nc.gpsimd.load_library(library_config.attn)

    const = ctx.enter_context(tc.tile_pool(name="const", bufs=1))
    expp = ctx.enter_context(tc.tile_pool(name="exps", bufs=3))
    ps_s = ctx.enter_context(tc.tile_pool(name="ps_s", bufs=3, space="PSUM"))
    ps_o = ctx.enter_context(tc.tile_pool(name="ps_o", bufs=2, space="PSUM"))

    # ---------------- loads ----------------
    xTe = const.tile([CE, HW], F32R)
    nc.sync.dma_start(xTe[:], xTe_d)
    yTe = const.tile([CE, HW], F32R)
    nc.sync.dma_start(yTe[:], yTe_d)
    wqe = const.tile([CE, D], F32R)
    nc.sync.dma_start(wqe[:], wqe_d)
    wke = const.tile([CE, D], F32R)
    nc.sync.dma_start(wke[:], wke_d)
    wve = const.tile([CE, D], F32R)
    nc.sync.dma_start(wve[:], wve_d)
    webe = const.tile([VW, C], F32)
    nc.sync.dma_start(webe[:], webe_d)
    sel = const.tile([D, D * NQB], F32R)
    nc.sync.dma_start(sel[:], sel_d)

    # ---------------- q/k projections (transposed layout) ----------------
    qT = const.tile([D, HW], F32)
    kT = const.tile([D, HW], F32)
    for dst, w, src in ((qT, wqe, xTe), (kT, wke, yTe)):
        for j in range(NQB):
            ps = ps_s.tile([D, QB], F32, tag="pss")
            nc.tensor.matmul(ps[:], w[:], src[:, ts(j, QB)], start=True, stop=True)
            nc.vector.tensor_copy(dst[:, ts(j, QB)], ps[:])

    # ---------------- v prep (row layout, bf16, ones col) ----------------
    vext = const.tile([KC, VW * NKC], BF16)
    nc.vector.memset(vext[:], 1.0)
    for c in range(NKC):
        ps = ps_s.tile([KC, D], F32, tag="pss")
        nc.tensor.matmul(ps[:], yTe[:, ts(c, KC)], wve[:], start=True, stop=True)
        nc.vector.tensor_copy(vext[:, c * VW:c * VW + D], ps[:])

    # ---------------- inverse norms (free layout, partition-packed) -------
    qTn = const.tile([D, HW], BF16)
    kTn = const.tile([D, HW], BF16)
    sq = const.tile([D, HW], F32R)  # squares scratch
    ssq_q = const.tile([NQB, QB], F32)
    ssq_k = const.tile([NQB, QB], F32)
    for src, ssq in ((qT, ssq_q), (kT, ssq_k)):
        nc.vector.tensor_mul(sq[:], src[:], src[:])
        ps = ps_s.tile([NQB, QB], F32, tag="pss")
        for j in range(NQB):
            nc.tensor.matmul(ps[:], sel[:, ts(j, D)], sq[:, ts(j, QB)],
                             start=(j == 0), stop=(j == NQB - 1))
        nc.vector.tensor_copy(ssq[:], ps[:])

    # invsqrt = newton(recip_accurate(sqrt(ssq)))
    inv_q = const.tile([NQB, QB], F32)
    inv_k = const.tile([NQB, QB], F32)
    scr = const.tile([NQB, QB], F32)
    scr2 = const.tile([NQB, QB], F32)
    for ssq, inv in ((ssq_q, inv_q), (ssq_k, inv_k)):
        nc.scalar.sqrt(scr[:], ssq[:])
        nc.vector.reciprocal_approx_accurate(inv[:], scr[:], scr2[:])
        nc.vector.tensor_mul(scr[:], inv[:], inv[:])
        nc.vector.tensor_mul(scr[:], scr[:], ssq[:])
        nc.vector.tensor_scalar(scr[:], scr[:], -0.5, 1.5,
                                mybir.AluOpType.mult, mybir.AluOpType.add)
        nc.vector.tensor_mul(inv[:], inv[:], scr[:])

    # replicate inv norms to D partitions; normalize into fp32r tiles
    rep = const.tile([D, HW], F32)
    invf = const.tile([1, HW], F32)
    for inv, src, dst in ((inv_q, qT, qTn), (inv_k, kT, kTn)):
        nc.sync.dma_start(invf[:], inv[:])  # [NQB, QB] -> [1, HW]
        if REPL == "pb":
            nc.gpsimd.partition_broadcast(rep[:], invf[:], channels=D)
        else:
            for p in range(D):
                nc.sync.dma_start(rep[p:p + 1, :], invf[:])
        nc.vector.tensor_mul(dst[:], src[:], rep[:])

    # ---------------- main attention loop ----------------
    # oTe rows 0-7: unnormalized numerator; row 8: softmax denominator
    # (after scaling by the replicated reciprocal, row 8 becomes den/den = 1,
    # which is exactly the ones-row the output projection needs for be/8).
    oTe = const.tile([VW, HW], F32)

    for j in range(NQB):
        po = ps_o.tile([VW, QB], F32, tag="pso")
        for g in range(NG):
            ps = ps_s.tile([KC, GRP * QB], F32, tag="pss")
            for u in range(GRP):
                c = g * GRP + u
                nc.tensor.matmul(ps[:, ts(u, QB)], kTn[:, ts(c, KC)],
                                 qTn[:, ts(j, QB)], start=True, stop=True)
            es = expp.tile([KC, GRP * QB], BF16, tag="es")
            nc.scalar.activation(es[:], ps[:], mybir.ActivationFunctionType.Exp)
            for u in range(GRP):
                c = g * GRP + u
                nc.tensor.matmul(po[:], vext[:, c * VW:(c + 1) * VW],
                                 es[:, ts(u, QB)],
                                 start=(g == 0 and u == 0),
                                 stop=(g == NG - 1 and u == GRP - 1))
        nc.vector.tensor_copy(oTe[:, ts(j, QB)], po[:])

    # ---------------- normalize + output projection ----------------
    # repack denominator row [1, HW] -> [NQB, QB] via DMA (partition crossing)
    den8 = const.tile([NQB, QB], F32)
    nc.sync.dma_start(den8[:], oTe[D:D + 1, :])
    invd = const.tile([NQB, QB], F32)
    scr3 = const.tile([NQB, QB], F32)
    nc.vector.reciprocal_approx_accurate(invd[:], den8[:], scr3[:])
    nc.sync.dma_start(invf[:], invd[:])
    repd = const.tile([VW, HW], F32)
    if REPL == "pb":
        nc.gpsimd.partition_broadcast(repd[:], invf[:], channels=VW)
    else:
        for p in range(VW):
            nc.sync.dma_start(repd[p:p + 1, :], invf[:])
    nc.vector.tensor_mul(oTe[:], oTe[:], repd[:])

    resT = const.tile([C, HW], F32)
    for j in range(NQB):
        ps = ps_s.tile([C, QB], F32, tag="pss")
        nc.tensor.matmul(ps[:], webe[:], oTe[:, ts(j, QB)], start=True, stop=True)
        nc.vector.tensor_copy(resT[:, ts(j, QB)], ps[:])
    nc.sync.dma_start(out_d, resT[:])


def _build():
    global _BUILT
    if _BUILT is not None:
        return _BUILT
    nc = bacc.Bacc("TRN2", target_bir_lowering=False, debug=False, num_devices=H)
    xTe_d = nc.dram_tensor("xTe", [CE, HW], F32R, kind="ExternalInput").ap()
    yTe_d = nc.dram_tensor("yTe", [CE, HW], F32R, kind="ExternalInput").ap()
    wqe_d = nc.dram_tensor("wqe", [CE, D], F32R, kind="ExternalInput").ap()
    wke_d = nc.dram_tensor("wke", [CE, D], F32R, kind="ExternalInput").ap()
    wve_d = nc.dram_tensor("wve", [CE, D], F32R, kind="ExternalInput").ap()
    webe_d = nc.dram_tensor("webe", [VW, C], F32, kind="ExternalInput").ap()
    sel_d = nc.dram_tensor("sel", [D, D * NQB], F32R, kind="ExternalInput").ap()
    out_d = nc.dram_tensor("resT", [C, HW], F32, kind="ExternalOutput").ap()
    with tile.TileContext(nc) as tc, ExitStack() as ctx:
        _body(ctx, tc, (xTe_d, yTe_d, wqe_d, wke_d, wve_d, webe_d, sel_d,
                        out_d[:]))
    nc.compile()
    _BUILT = nc
    return nc


def make_in_maps(x, y, Wq, bq, Wkv, bkv, We, be):
    x, y, Wq, bq, Wkv, bkv, We, be = (
        np.asarray(a, np.float32) for a in (x, y, Wq, bq, Wkv, bkv, We, be))
    ones = np.ones((1, HW), np.float32)
    xTe = np.ascontiguousarray(np.vstack([x[0].T, ones]))
    yTe = np.ascontiguousarray(np.vstack([y[0].T, ones]))
    sel = np.zeros((D, D * NQB), np.float32)
    for j in range(NQB):
        sel[:, D * j + j] = 1.0
    in_maps = []
    for h in range(H):
        sl = slice(h * D, (h + 1) * D)
        slv = slice(C + h * D, C + (h + 1) * D)
        in_maps.append({
            "xTe": xTe,
            "yTe": yTe,
            "wqe": np.ascontiguousarray(np.vstack([Wq[:, sl], bq[None, sl]])),
            "wke": np.ascontiguousarray(np.vstack([Wkv[:, sl], bkv[None, sl]])),
            "wve": np.ascontiguousarray(np.vstack([Wkv[:, slv], bkv[None, slv]])),
            "webe": np.ascontiguousarray(np.vstack([We[sl, :], be[None, :] / H])),
            "sel": sel,
        })
    return in_maps


def kernel(x, y, Wq, bq, Wkv, bkv, We, be):
    global LAST_RESULTS
    nc = _build()
    in_maps = make_in_maps(x, y, Wq, bq, Wkv, bkv, We, be)
    res = run_bass_kernel_spmd(nc, in_maps, core_ids=list(range(H)), trace=TRACE)
    LAST_RESULTS = res
    acc = np.zeros((C, HW), np.float64)
    for r in res.results:
        acc += r["resT"]
    return np.ascontiguousarray(acc.T[None]).astype(np.float32)


# revision 16
# speedup vs baseline: 1.4934x; 1.4027x over previous
"""Trainium2 Bass kernel for cross-attention (cosine-normalized, 8 heads).

Reference computation (full inputs x,y [1,4096,64]):
  q = x@Wq+bq ; k,v = split(y@Wkv+bkv) ; per head (8 heads, dim 8):
  attn = softmax(l2norm(q) @ l2norm(k)^T) ; out = attn@v
  result = concat_heads(out) @ We + be

Sharding: one head per NeuronCore (8 heads / 8 cores), SPMD program with
per-core weight slices. Each core returns resT_h = (out_h @ We_h + be/8)^T
as [64, 4096]; the host sums over cores and transposes.

Device algorithm per core (head h):
  - host passes xTe=[x^T; 1] [65,4096] (ones row folds biases into matmuls),
    yTe likewise, weight slices with bias rows, and a selector constant.
  - qT [8,4096] = Wqe^T @ xTe ; kT likewise (PE, K=65, fp32r).
  - norms in free layout: squares on DVE; selector matmuls pack per-block
    column sums into one [8,512] psum; invsqrt = newton(recip(sqrt)));
    flatten via DMA, replicate via gpsimd partition_broadcast; qT/kT
    normalized into fp32r tiles.
  - v chunks [128,8] = yTe_chunk^T @ Wve stored bf16 with ones column
    (vext [128, 9*32]); the ones column produces the softmax denominator.
  - main loop (8 q-blocks x 16 chunk-groups):
      scores^T [128,1024] = kTn_chunks^T qTn_block (fp32r, two matmuls)
      expS bf16 = Exp(scores) on ScalarE (cosine scores in [-1,1]: no
        max-subtraction needed)
      po [9,512] += vext_chunk^T @ expS  (rows 0-7 numerator, row 8 denom)
  - tail: invden = recip_accurate(den); oTe (incl. denom row) scaled by
    broadcast invden -> row 8 becomes exactly 1 = bias row for the final
    fp32 projection resT = WeBe^T @ oTe; DMA out.
"""

import sys

import numpy as np

for _p in ("/opt/trn_rl_repo",):
    if _p not in sys.path:
        sys.path.insert(0, _p)

from contextlib import ExitStack

import concourse.bass as bass
import concourse.tile as tile
from concourse import bacc, mybir
from concourse.bass import ts
from concourse.bass_utils import run_bass_kernel_spmd

F32 = mybir.dt.float32
F32R = mybir.dt.float32r
BF16 = mybir.dt.bfloat16

HW = 4096          # sequence length
C = 64             # model dim
H = 8              # heads
D = 8              # head dim
CE = C + 1         # +ones row for bias folding
QB = 512           # q block
NQB = HW // QB     # 8
KC = 128           # k chunk
NKC = HW // KC     # 32
GRP = 2            # k-chunks per exp/ACT group
NG = NKC // GRP    # 16
VW = D + 1         # v + ones column

REPL = "dma"        # inv replication: gpsimd partition_broadcast vs row DMAs

_BUILT = None
TRACE = False
LAST_RESULTS = None


def _body(ctx, tc, dram):
    nc = tc.nc
    xTe_d, yTe_d, wqe_d, wke_d, wve_d, webe_d, sel_d, out_d = dram

    if REPL == "pb":
        from concourse import library_config
        nc.gpsimd.load_library(library_config.attn)

    const = ctx.enter_context(tc.tile_pool(name="const", bufs=1))
    expp = ctx.enter_context(tc.tile_pool(name="exps", bufs=3))
    ps_s = ctx.enter_context(tc.tile_pool(name="ps_s", bufs=3, space="PSUM"))
    ps_o = ctx.enter_context(tc.tile_pool(name="ps_o", bufs=2, space="PSUM"))

    # ---------------- loads ----------------
    xTe = const.tile([CE, HW], F32R)
    nc.sync.dma_start(xTe[:], xTe_d)
    yTe = const.tile([CE, HW], F32R)
    nc.sync.dma_start(yTe[:], yTe_d)
    wqe = const.tile([CE, D], F32R)
    nc.sync.dma_start(wqe[:], wqe_d)
    wke = const.tile([CE, D], F32R)
    nc.sync.dma_start(wke[:], wke_d)
    wve = const.tile([CE, D], F32R)
    nc.sync.dma_start(wve[:], wve_d)
    webe = const.tile([VW, C], F32)
    nc.sync.dma_start(webe[:], webe_d)
    sel = const.tile([D, D * NQB], F32R)
    nc.sync.dma_start(sel[:], sel_d)

    # ---------------- q/k projections (transposed layout) ----------------
    qT = const.tile([D, HW], F32)
    kT = const.tile([D, HW], F32)
    for dst, w, src in ((qT, wqe, xTe), (kT, wke, yTe)):
        for j in range(NQB):
            ps = ps_s.tile([D, QB], F32, tag="pss")
            nc.tensor.matmul(ps[:], w[:], src[:, ts(j, QB)], start=True, stop=True)
            nc.vector.tensor_copy(dst[:, ts(j, QB)], ps[:])

    # ---------------- v prep (row layout, bf16, ones col) ----------------
    vext = const.tile([KC, VW * NKC], BF16)
    nc.vector.memset(vext[:], 1.0)
    for c in range(NKC):
        ps = ps_s.tile([KC, D], F32, tag="pss")
        nc.tensor.matmul(ps[:], yTe[:, ts(c, KC)], wve[:], start=True, stop=True)
        nc.vector.tensor_copy(vext[:, c * VW:c * VW + D], ps[:])

    # ---------------- inverse norms (free layout, partition-packed) -------
    # qTn/kTn are [128, HW] with rows D..127 zero: K=128 matmuls run at the
    # full 2.4 GHz PE clock while K=8 ones stay cold-throttled (measured).
    qTn = const.tile([KC, HW], BF16)
    nc.vector.memset(qTn[:], 0.0)
    kTn = const.tile([KC, HW], BF16)
    nc.vector.memset(kTn[:], 0.0)
    sq = const.tile([D, HW], F32R)  # squares scratch
    ssq_q = const.tile([NQB, QB], F32)
    ssq_k = const.tile([NQB, QB], F32)
    for src, ssq in ((qT, ssq_q), (kT, ssq_k)):
        nc.vector.tensor_mul(sq[:], src[:], src[:])
        ps = ps_s.tile([NQB, QB], F32, tag="pss")
        for j in range(NQB):
            nc.tensor.matmul(ps[:], sel[:, ts(j, D)], sq[:, ts(j, QB)],
                             start=(j == 0), stop=(j == NQB - 1))
        nc.vector.tensor_copy(ssq[:], ps[:])

    # invsqrt = newton(recip_accurate(sqrt(ssq)))
    inv_q = const.tile([NQB, QB], F32)
    inv_k = const.tile([NQB, QB], F32)
    scr = const.tile([NQB, QB], F32)
    scr2 = const.tile([NQB, QB], F32)
    for ssq, inv in ((ssq_q, inv_q), (ssq_k, inv_k)):
        nc.scalar.sqrt(scr[:], ssq[:])
        nc.vector.reciprocal_approx_accurate(inv[:], scr[:], scr2[:])
        nc.vector.tensor_mul(scr[:], inv[:], inv[:])
        nc.vector.tensor_mul(scr[:], scr[:], ssq[:])
        nc.vector.tensor_scalar(scr[:], scr[:], -0.5, 1.5,
                                mybir.AluOpType.mult, mybir.AluOpType.add)
        nc.vector.tensor_mul(inv[:], inv[:], scr[:])

    # replicate inv norms to D partitions; normalize into fp32r tiles
    rep = const.tile([D, HW], F32)
    invf = const.tile([1, HW], F32)
    for inv, src, dst in ((inv_q, qT, qTn), (inv_k, kT, kTn)):
        nc.sync.dma_start(invf[:], inv[:])  # [NQB, QB] -> [1, HW]
        if REPL == "pb":
            nc.gpsimd.partition_broadcast(rep[:], invf[:], channels=D)
        else:
            for p in range(D):
                nc.sync.dma_start(rep[p:p + 1, :], invf[:])
        nc.vector.tensor_mul(dst[0:D, :], src[:], rep[:])

    # ---------------- main attention loop ----------------
    # oTe rows 0-7: unnormalized numerator; row 8: softmax denominator
    # (after scaling by the replicated reciprocal, row 8 becomes den/den = 1,
    # which is exactly the ones-row the output projection needs for be/8).
    oTe = const.tile([VW, HW], F32)

    for j in range(NQB):
        po = ps_o.tile([VW, QB], F32, tag="pso")
        for g in range(NG):
            ps = ps_s.tile([KC, GRP * QB], F32, tag="pss")
            for u in range(GRP):
                c = g * GRP + u
                nc.tensor.matmul(ps[:, ts(u, QB)], kTn[:, ts(c, KC)],
                                 qTn[:, ts(j, QB)], start=True, stop=True)
            es = expp.tile([KC, GRP * QB], BF16, tag="es")
            nc.scalar.activation(es[:], ps[:], mybir.ActivationFunctionType.Exp)
            for u in range(GRP):
                c = g * GRP + u
                nc.tensor.matmul(po[:], vext[:, c * VW:(c + 1) * VW],
                                 es[:, ts(u, QB)],
                                 start=(g == 0 and u == 0),
                                 stop=(g == NG - 1 and u == GRP - 1))
        nc.vector.tensor_copy(oTe[:, ts(j, QB)], po[:])

    # ---------------- normalize + output projection ----------------
    # repack denominator row [1, HW] -> [NQB, QB] via DMA (partition crossing)
    den8 = const.tile([NQB, QB], F32)
    nc.sync.dma_start(den8[:], oTe[D:D + 1, :])
    invd = const.tile([NQB, QB], F32)
    scr3 = const.tile([NQB, QB], F32)
    nc.vector.reciprocal_approx_accurate(invd[:], den8[:], scr3[:])
    nc.sync.dma_start(invf[:], invd[:])
    repd = const.tile([VW, HW], F32)
    if REPL == "pb":
        nc.gpsimd.partition_broadcast(repd[:], invf[:], channels=VW)
    else:
        for p in range(VW):
            nc.sync.dma_start(repd[p:p + 1, :], invf[:])
    nc.vector.tensor_mul(oTe[:], oTe[:], repd[:])

    resT = const.tile([C, HW], F32)
    for j in range(NQB):
        ps = ps_s.tile([C, QB], F32, tag="pss")
        nc.tensor.matmul(ps[:], webe[:], oTe[:, ts(j, QB)], start=True, stop=True)
        nc.vector.tensor_copy(resT[:, ts(j, QB)], ps[:])
    nc.sync.dma_start(out_d, resT[:])


def _build():
    global _BUILT
    if _BUILT is not None:
        return _BUILT
    nc = bacc.Bacc("TRN2", target_bir_lowering=False, debug=False, num_devices=H)
    xTe_d = nc.dram_tensor("xTe", [CE, HW], F32R, kind="ExternalInput").ap()
    yTe_d = nc.dram_tensor("yTe", [CE, HW], F32R, kind="ExternalInput").ap()
    wqe_d = nc.dram_tensor("wqe", [CE, D], F32R, kind="ExternalInput").ap()
    wke_d = nc.dram_tensor("wke", [CE, D], F32R, kind="ExternalInput").ap()
    wve_d = nc.dram_tensor("wve", [CE, D], F32R, kind="ExternalInput").ap()
    webe_d = nc.dram_tensor("webe", [VW, C], F32, kind="ExternalInput").ap()
    sel_d = nc.dram_tensor("sel", [D, D * NQB], F32R, kind="ExternalInput").ap()
    out_d = nc.dram_tensor("resT", [C, HW], F32, kind="ExternalOutput").ap()
    with tile.TileContext(nc) as tc, ExitStack() as ctx:
        _body(ctx, tc, (xTe_d, yTe_d, wqe_d, wke_d, wve_d, webe_d, sel_d,
                        out_d[:]))
    nc.compile()
    _BUILT = nc
    return nc


def make_in_maps(x, y, Wq, bq, Wkv, bkv, We, be):
    x, y, Wq, bq, Wkv, bkv, We, be = (
        np.asarray(a, np.float32) for a in (x, y, Wq, bq, Wkv, bkv, We, be))
    ones = np.ones((1, HW), np.float32)
    xTe = np.ascontiguousarray(np.vstack([x[0].T, ones]))
    yTe = np.ascontiguousarray(np.vstack([y[0].T, ones]))
    sel = np.zeros((D, D * NQB), np.float32)
    for j in range(NQB):
        sel[:, D * j + j] = 1.0
    in_maps = []
    for h in range(H):
        sl = slice(h * D, (h + 1) * D)
        slv = slice(C + h * D, C + (h + 1) * D)
        in_maps.append({
            "xTe": xTe,
            "yTe": yTe,
            "wqe": np.ascontiguousarray(np.vstack([Wq[:, sl], bq[None, sl]])),
            "wke": np.ascontiguousarray(np.vstack([Wkv[:, sl], bkv[None, sl]])),
            "wve": np.ascontiguousarray(np.vstack([Wkv[:, slv], bkv[None, slv]])),
            "webe": np.ascontiguousarray(np.vstack([We[sl, :], be[None, :] / H])),
            "sel": sel,
        })
    return in_maps


def kernel(x, y, Wq, bq, Wkv, bkv, We, be):
    global LAST_RESULTS
    nc = _build()
    in_maps = make_in_maps(x, y, Wq, bq, Wkv, bkv, We, be)
    res = run_bass_kernel_spmd(nc, in_maps, core_ids=list(range(H)), trace=TRACE)
    LAST_RESULTS = res
    acc = np.zeros((C, HW), np.float64)
    for r in res.results:
        acc += r["resT"]
    return np.ascontiguousarray(acc.T[None]).astype(np.float32)
